# revision 14
# baseline (speedup 1.0000x reference)
"""Trainium2 Bass kernel for nn_Cross_Attn (B=8, N=1024, C=512).

Sharding: data-parallel over batch B across the 8 NeuronCores (one batch
element per core); the three [C,C] projection weights and beta are
replicated to every core.

Per-core math (batch element b):
  q = x @ Wq.T + bq ; k = y @ Wk.T + bk ; v* = {x,y} @ Wv.T + bv
  E[i,j] = q[i].k[j]
  out_x = beta^2 * softmax_row(E) @ vx + (1+beta) * vx     (reference applies
          the "beta*out+vx" residual twice)
  out_y = softmax_row(E.T) @ vy

Both softmaxes are served by ONE exponentiation with a single global shift
(gmax = max E): P = exp(E - gmax).  Row sums of P give the out_x
normalizer; row sums of P.T give the out_y normalizer, and P / P.T are
exactly the stationary matmul operands the two attention contractions
need.  Biases are folded in via "attention rows sum to 1":
  attn @ (v_nb + 1*bv) = attn @ v_nb + bv.

Precision: projections for q/k and the QK^T energy run in fp32 on the PE
(softmax logits have std ~22, bf16 there would corrupt the attention
weights); the post-softmax value path runs in bf16 (errors are averaged
by the convex attention combination).
"""

import os

import numpy as np
import ml_dtypes

import concourse.bacc as bacc
import concourse.bass as bass
import concourse.bass_isa as bass_isa
import concourse.mybir as mybir
import concourse.tile as tile
from concourse.bass_utils import run_bass_kernel_spmd

B, N, C = 8, 1024, 512
P = 128
NCH = N // P  # 8 chunks of token rows
CCH = C // P  # 4 chunks of channels
F32 = mybir.dt.float32
BF16 = mybir.dt.bfloat16
AX = mybir.AxisListType.X
ALU = mybir.AluOpType
AF = mybir.ActivationFunctionType
OFF = 35.0  # softmax range-centering offset (see _emit)
F32R = mybir.dt.float32r
# dtype used for the q/k/E/v projection matmul operands: float32 is exact
# but runs the PE at 1/4 rate; float32r uses the replicated-fp32 PE mode
# (full rate for free-dim >= 256) with reduced mantissa on hardware.
MM_DT = F32R if os.environ.get("KERNEL_MM_DT", "f32r") == "f32r" else F32



def _emit(nc, tc, d):
    """Emit the per-core kernel IR. `d` maps dram tensor name -> AP."""
    from contextlib import ExitStack

    with ExitStack() as ctx:
        cpool = ctx.enter_context(tc.tile_pool(name="const", bufs=1))
        psum_mm = ctx.enter_context(tc.tile_pool(name="psum_mm", bufs=4, space="PSUM"))
        out_pool = ctx.enter_context(tc.tile_pool(name="outs", bufs=4))

        # ---- constant / input loads -------------------------------------
        # One big sprayed DMA per tensor (partitions fan out over the DMA
        # ports); q-path tensors first so the first matmul group can start,
        # with the y/k-path and v-path streams on separate issue queues.
        xT_t = cpool.tile([P, CCH, N], MM_DT, name="xT_t")
        yT_t = cpool.tile([P, CCH, N], MM_DT, name="yT_t")
        wq_t = cpool.tile([P, CCH, C], MM_DT, name="wq_t")
        wk_t = cpool.tile([P, CCH, C], MM_DT, name="wk_t")
        wv_t = cpool.tile([P, CCH, C], MM_DT, name="wv_t")
        nc.sync.dma_start(wq_t[:], d["wqT"].rearrange("(c p) n -> p c n", p=P))
        nc.gpsimd.dma_start(wk_t[:], d["wkT"].rearrange("(c p) n -> p c n", p=P))
        nc.sync.dma_start(xT_t[:], d["xT"].rearrange("(c p) n -> p c n", p=P))
        nc.gpsimd.dma_start(yT_t[:], d["yT"].rearrange("(c p) n -> p c n", p=P))
        nc.scalar.dma_start(wv_t[:], d["wvT"].rearrange("(c p) n -> p c n", p=P))
        xT = [xT_t[:, c, :] for c in range(CCH)]
        yT = [yT_t[:, c, :] for c in range(CCH)]
        wq = [wq_t[:, c, :] for c in range(CCH)]
        wk = [wk_t[:, c, :] for c in range(CCH)]
        wv = [wv_t[:, c, :] for c in range(CCH)]
        bq_sb = cpool.tile([P, CCH], F32, name="bq_sb")
        bk_sb = cpool.tile([P, CCH], F32, name="bk_sb")
        nc.scalar.dma_start(bq_sb[:], d["bq"].rearrange("(c p) -> p c", p=P))
        nc.scalar.dma_start(bk_sb[:], d["bk"].rearrange("(c p) -> p c", p=P))
        bvb = cpool.tile([P, C], F32, name="bvb")
        bvxb = cpool.tile([P, C], F32, name="bvxb")
        consts = cpool.tile([P, 2], F32, name="consts")
        ident = cpool.tile([P, P], BF16, name="ident")
        nc.scalar.dma_start(bvb[:], d["bvb"][:])
        nc.scalar.dma_start(bvxb[:], d["bvxb"][:])
        nc.scalar.dma_start(consts[:], d["consts"][:])
        nc.scalar.dma_start(ident[:], d["ident"][:])

        # stats tiles
        rmax = cpool.tile([P, 2 * NCH], F32, name="rmax")
        rmax_x = cpool.tile([P, NCH], F32, name="rmax_x")
        rmneg = cpool.tile([P, NCH], F32, name="rmneg")
        gmax0 = cpool.tile([P, 1], F32, name="gmax0")
        gmax1 = cpool.tile([P, 1], F32, name="gmax1")
        gneg = cpool.tile([P, 1], F32, name="gneg")
        ffp = cpool.tile([P, NCH], F32, name="ffp")
        fbf = cpool.tile([P, NCH], BF16, name="fbf")
        zx = cpool.tile([P, NCH], F32, name="zx")
        rx = cpool.tile([P, NCH], F32, name="rx")
        sx = cpool.tile([P, NCH], F32, name="sx")
        zy = cpool.tile([P, NCH], F32, name="zy")
        sy = cpool.tile([P, NCH], F32, name="sy")

        e_pool = ctx.enter_context(tc.tile_pool(name="epool", bufs=1))
        E = [e_pool.tile([P, N], F32, name=f"E{i}") for i in range(NCH)]
        v_pool = ctx.enter_context(tc.tile_pool(name="vpool", bufs=1))
        vxbf = [v_pool.tile([P, C], BF16, name=f"vxbf{r}") for r in range(NCH)]
        vybf = [v_pool.tile([P, C], BF16, name=f"vybf{r}") for r in range(NCH)]
        vxb = [v_pool.tile([P, C], F32, name=f"vxb{r}") for r in range(NCH)]

        # ---- q/k projections (fp32), transposed layout [c_out, i] -------
        with tc.tile_pool(name="qkpool", bufs=1) as qk_pool:
            qT = [qk_pool.tile([P, N], MM_DT, name=f"qT{c}") for c in range(CCH)]
            kT = [qk_pool.tile([P, N], MM_DT, name=f"kT{c}") for c in range(CCH)]
            for co in range(CCH):
                for h in range(2):
                    ps = psum_mm.tile([P, 512], F32, tag="mmps")
                    for ci in range(CCH):
                        nc.tensor.matmul(
                            ps[:],
                            wq[ci][:, co * P : (co + 1) * P],
                            xT[ci][:, h * 512 : (h + 1) * 512],
                            start=(ci == 0),
                            stop=(ci == CCH - 1),
                        )
                    if h == 0:
                        nc.scalar.activation(
                            qT[co][:, h * 512 : (h + 1) * 512], ps[:],
                            AF.Identity, bias=bq_sb[:, co : co + 1],
                        )
                    else:
                        nc.vector.tensor_scalar_add(
                            qT[co][:, h * 512 : (h + 1) * 512], ps[:],
                            bq_sb[:, co : co + 1],
                        )
                for h in range(2):
                    ps = psum_mm.tile([P, 512], F32, tag="mmps")
                    for ci in range(CCH):
                        nc.tensor.matmul(
                            ps[:],
                            wk[ci][:, co * P : (co + 1) * P],
                            yT[ci][:, h * 512 : (h + 1) * 512],
                            start=(ci == 0),
                            stop=(ci == CCH - 1),
                        )
                    if h == 0:
                        nc.scalar.activation(
                            kT[co][:, h * 512 : (h + 1) * 512], ps[:],
                            AF.Identity, bias=bk_sb[:, co : co + 1],
                        )
                    else:
                        nc.vector.tensor_scalar_add(
                            kT[co][:, h * 512 : (h + 1) * 512], ps[:],
                            bk_sb[:, co : co + 1],
                        )

            # ---- energy E = q @ k.T (fp32), + row maxes -----------------
            for i in range(NCH):
                for h in range(2):
                    ps = psum_mm.tile([P, 512], F32, tag="mmps")
                    for cc in range(CCH):
                        nc.tensor.matmul(
                            ps[:],
                            qT[cc][:, i * P : (i + 1) * P],
                            kT[cc][:, h * 512 : (h + 1) * 512],
                            start=(cc == 0),
                            stop=(cc == CCH - 1),
                        )
                    nc.scalar.copy(E[i][:, h * 512 : (h + 1) * 512], ps[:])
                    nc.vector.reduce_max(
                        rmax[:, 2 * i + h : 2 * i + h + 1], ps[:], axis=AX
                    )

        # ---- softmax shift stats ----------------------------------------
        # P'[i,j] = exp(E[i,j] - rmax[i] + OFF)   (bf16, row-softmax scaled
        #           by e^OFF so cross-extreme entries stay in normal range)
        # f[i]    = exp(rmax[i] - gmax + OFF)     (folds the row shift back
        #           out for the column-softmax / out_y path)
        nc.vector.reduce_max(
            rmax_x[:], rmax[:].rearrange("p (i h) -> p i h", h=2), axis=AX
        )
        nc.vector.tensor_scalar(
            rmneg[:], rmax_x[:], -1.0, OFF, op0=ALU.mult, op1=ALU.add
        )
        nc.vector.reduce_max(gmax0[:], rmax[:], axis=AX)
        nc.gpsimd.partition_all_reduce(
            gmax1[:], gmax0[:], channels=P, reduce_op=bass_isa.ReduceOp.max
        )
        nc.vector.tensor_scalar(
            gneg[:], gmax1[:], -1.0, OFF, op0=ALU.mult, op1=ALU.add
        )
        nc.scalar.activation(fbf[:], rmax_x[:], AF.Exp, bias=gneg[:, 0:1])
        nc.vector.tensor_copy(ffp[:], fbf[:])  # f32 view for DVE scalar use

        # ---- value projections (fp32 in, bf16 rhs copies) ----------------
        # Emitted after E so the PE stays busy during the exp phase.
        for r in range(NCH):
            ps = psum_mm.tile([P, 512], F32, tag="mmps")
            for ci in range(CCH):
                nc.tensor.matmul(
                    ps[:], xT[ci][:, r * P : (r + 1) * P], wv[ci][:],
                    start=(ci == 0), stop=(ci == CCH - 1),
                )
            nc.scalar.copy(vxbf[r][:], ps[:])
            # vxb = (1+beta)*vx_nb + (1+beta+beta^2)*bv
            nc.vector.scalar_tensor_tensor(
                vxb[r][:], ps[:], consts[:, 0:1], bvxb[:],
                op0=ALU.mult, op1=ALU.add,
            )
        for r in range(NCH):
            ps = psum_mm.tile([P, 512], F32, tag="mmps")
            for ci in range(CCH):
                nc.tensor.matmul(
                    ps[:], yT[ci][:, r * P : (r + 1) * P], wv[ci][:],
                    start=(ci == 0), stop=(ci == CCH - 1),
                )
            # vy' = f[i] * vy_nb  (per-partition scale)
            nc.vector.tensor_scalar_mul(vybf[r][:], ps[:], ffp[:, r : r + 1])

        # ---- exp + out_x normalizers ------------------------------------
        p_pool = ctx.enter_context(tc.tile_pool(name="ppool", bufs=1))
        praw = [p_pool.tile([P, N], BF16, name=f"praw{i}") for i in range(NCH)]
        prawT = [p_pool.tile([P, N], BF16, name=f"prawT{j}") for j in range(NCH)]

        for i in range(NCH):
            nc.scalar.activation(
                praw[i][:], E[i][:], AF.Exp,
                bias=rmneg[:, i : i + 1], scale=1.0,
                accum_out=zx[:, i : i + 1],
            )
        nc.vector.reciprocal(rx[:], zx[:])
        nc.vector.tensor_scalar_mul(sx[:], rx[:], consts[:, 1:2])  # * beta^2

        # out_y normalizers: zy[j] = sum_i P'[i,j] * f[i] via f-vector matmuls
        zy_ps_pool = ctx.enter_context(
            tc.tile_pool(name="zyps", bufs=2, space="PSUM")
        )
        for jt in range(NCH):
            zps = zy_ps_pool.tile([P, 1], F32, tag="zyps")
            for ic in range(NCH):
                nc.tensor.matmul(
                    zps[:], praw[ic][:, jt * P : (jt + 1) * P],
                    fbf[:, ic : ic + 1],
                    start=(ic == 0), stop=(ic == NCH - 1),
                )
            nc.scalar.copy(zy[:, jt : jt + 1], zps[:])
        nc.vector.reciprocal(sy[:], zy[:])

        # ---- transposes + attention matmuls -----------------------------
        psum_tr = ctx.enter_context(tc.tile_pool(name="psum_tr", bufs=2, space="PSUM"))
        for t in range(NCH):
            # transpose row-chunk t of P into column pieces of prawT
            for jc in range(NCH):
                pst = psum_tr.tile([P, P], BF16, tag="trps")
                nc.tensor.transpose(
                    pst[:], praw[t][:, jc * P : (jc + 1) * P], ident[:]
                )
                nc.vector.tensor_copy(prawT[jc][:, t * P : (t + 1) * P], pst[:])

            # out_y tile t: contraction over i via praw directly
            ps = psum_mm.tile([P, 512], F32, tag="mmps")
            for ic in range(NCH):
                nc.tensor.matmul(
                    ps[:], praw[ic][:, t * P : (t + 1) * P], vybf[ic][:],
                    start=(ic == 0), stop=(ic == NCH - 1),
                )
            oy = out_pool.tile([P, C], F32, tag="oy")
            nc.vector.scalar_tensor_tensor(
                oy[:], ps[:], sy[:, t : t + 1], bvb[:],
                op0=ALU.mult, op1=ALU.add,
            )
            nc.sync.dma_start(d["out_y"][t * P : (t + 1) * P, :], oy[:])

            # out_x tile t: needs prawT[jc][:, t*P:...] for all jc = the
            # transposes of this t-iteration
            ps = psum_mm.tile([P, 512], F32, tag="mmps")
            for jc in range(NCH):
                nc.tensor.matmul(
                    ps[:], prawT[jc][:, t * P : (t + 1) * P], vxbf[jc][:],
                    start=(jc == 0), stop=(jc == NCH - 1),
                )
            ox = out_pool.tile([P, C], F32, tag="ox")
            nc.vector.scalar_tensor_tensor(
                ox[:], ps[:], sx[:, t : t + 1], vxb[t][:],
                op0=ALU.mult, op1=ALU.add,
            )
            nc.sync.dma_start(d["out_x"][t * P : (t + 1) * P, :], ox[:])


_CACHE = {}


def _build():
    if "nc" in _CACHE:
        return _CACHE["nc"]
    nc = bacc.Bacc(
        "TRN2", target_bir_lowering=False, debug=False, enable_asserts=False,
        num_devices=B,
    )
    d = {}
    d["xT"] = nc.dram_tensor("xT", [C, N], MM_DT, kind="ExternalInput").ap()
    d["yT"] = nc.dram_tensor("yT", [C, N], MM_DT, kind="ExternalInput").ap()
    d["wqT"] = nc.dram_tensor("wqT", [C, C], MM_DT, kind="ExternalInput").ap()
    d["wkT"] = nc.dram_tensor("wkT", [C, C], MM_DT, kind="ExternalInput").ap()
    d["wvT"] = nc.dram_tensor("wvT", [C, C], MM_DT, kind="ExternalInput").ap()
    d["bq"] = nc.dram_tensor("bq", [C], F32, kind="ExternalInput").ap()
    d["bk"] = nc.dram_tensor("bk", [C], F32, kind="ExternalInput").ap()
    d["bvb"] = nc.dram_tensor("bvb", [P, C], F32, kind="ExternalInput").ap()
    d["bvxb"] = nc.dram_tensor("bvxb", [P, C], F32, kind="ExternalInput").ap()
    d["consts"] = nc.dram_tensor("consts", [P, 2], F32, kind="ExternalInput").ap()
    d["ident"] = nc.dram_tensor("ident", [P, P], BF16, kind="ExternalInput").ap()
    d["out_x"] = nc.dram_tensor("out_x", [N, C], F32, kind="ExternalOutput").ap()
    d["out_y"] = nc.dram_tensor("out_y", [N, C], F32, kind="ExternalOutput").ap()

    with tile.TileContext(nc) as tc:
        _emit(nc, tc, d)
    nc.compile()
    _CACHE["nc"] = nc
    return nc


LAST_EXEC_NS = None


def kernel(x, y, Wq, bq, Wk, bk, Wv, bv, beta):
    global LAST_EXEC_NS
    nc = _build()

    x = np.asarray(x, np.float32)
    y = np.asarray(y, np.float32)
    beta_f = float(np.asarray(beta).reshape(-1)[0])
    c1 = 1.0 + beta_f
    c2 = beta_f * beta_f
    wqT = np.ascontiguousarray(np.asarray(Wq, np.float32).T)
    wkT = np.ascontiguousarray(np.asarray(Wk, np.float32).T)
    wvT = np.ascontiguousarray(np.asarray(Wv, np.float32).T)
    bq = np.ascontiguousarray(np.asarray(bq, np.float32))
    bk = np.ascontiguousarray(np.asarray(bk, np.float32))
    bv = np.asarray(bv, np.float32)
    bvb = np.tile(bv[None, :], (P, 1))
    bvxb = np.tile(((1.0 + beta_f + beta_f * beta_f) * bv)[None, :], (P, 1))
    consts = np.tile(np.array([[c1, c2]], np.float32), (P, 1))
    ident = np.eye(P, dtype=ml_dtypes.bfloat16)

    shared = {
        "wqT": wqT, "wkT": wkT, "wvT": wvT, "bq": bq, "bk": bk,
        "bvb": np.ascontiguousarray(bvb), "bvxb": np.ascontiguousarray(bvxb),
        "consts": np.ascontiguousarray(consts), "ident": ident,
    }
    in_maps = []
    for b in range(B):
        m = dict(shared)
        m["xT"] = np.ascontiguousarray(x[b].T)
        m["yT"] = np.ascontiguousarray(y[b].T)
        in_maps.append(m)

    trace = os.environ.get("KERNEL_TRACE", "0") == "1"
    res = run_bass_kernel_spmd(nc, in_maps, core_ids=list(range(B)), trace=trace)
    LAST_EXEC_NS = res.exec_time_ns

    out_x = np.stack([np.asarray(res.results[b]["out_x"]) for b in range(B)])
    out_y = np.stack([np.asarray(res.results[b]["out_y"]) for b in range(B)])
    return out_x, out_y


# revision 16
# speedup vs baseline: 1.0476x; 1.0476x over previous
"""Trainium2 Bass kernel for nn_Cross_Attn (B=8, N=1024, C=512).

Sharding: data-parallel over batch B across the 8 NeuronCores (one batch
element per core); the three [C,C] projection weights and beta are
replicated to every core.

Per-core math (batch element b):
  q = x @ Wq.T + bq ; k = y @ Wk.T + bk ; v* = {x,y} @ Wv.T + bv
  E[i,j] = q[i].k[j]
  out_x = beta^2 * softmax_row(E) @ vx + (1+beta) * vx     (reference applies
          the "beta*out+vx" residual twice)
  out_y = softmax_row(E.T) @ vy

Both softmaxes are served by ONE exponentiation with a single global shift
(gmax = max E): P = exp(E - gmax).  Row sums of P give the out_x
normalizer; row sums of P.T give the out_y normalizer, and P / P.T are
exactly the stationary matmul operands the two attention contractions
need.  Biases are folded in via "attention rows sum to 1":
  attn @ (v_nb + 1*bv) = attn @ v_nb + bv.

Precision: projections for q/k and the QK^T energy run in fp32 on the PE
(softmax logits have std ~22, bf16 there would corrupt the attention
weights); the post-softmax value path runs in bf16 (errors are averaged
by the convex attention combination).
"""

import os

import numpy as np
import ml_dtypes

import concourse.bacc as bacc
import concourse.bass as bass
import concourse.bass_isa as bass_isa
import concourse.mybir as mybir
import concourse.tile as tile
from concourse.bass_utils import run_bass_kernel_spmd

B, N, C = 8, 1024, 512
P = 128
NCH = N // P  # 8 chunks of token rows
CCH = C // P  # 4 chunks of channels
F32 = mybir.dt.float32
BF16 = mybir.dt.bfloat16
AX = mybir.AxisListType.X
ALU = mybir.AluOpType
AF = mybir.ActivationFunctionType
OFF = 35.0  # softmax range-centering offset (see _emit)
F32R = mybir.dt.float32r
# dtype used for the q/k/E/v projection matmul operands: float32 is exact
# but runs the PE at 1/4 rate; float32r uses the replicated-fp32 PE mode
# (full rate for free-dim >= 256) with reduced mantissa on hardware.
MM_DT = F32R if os.environ.get("KERNEL_MM_DT", "f32r") == "f32r" else F32



def _emit(nc, tc, d):
    """Emit the per-core kernel IR. `d` maps dram tensor name -> AP."""
    from contextlib import ExitStack

    with ExitStack() as ctx:
        cpool = ctx.enter_context(tc.tile_pool(name="const", bufs=1))
        psum_mm = ctx.enter_context(tc.tile_pool(name="psum_mm", bufs=4, space="PSUM"))
        out_pool = ctx.enter_context(tc.tile_pool(name="outs", bufs=4))

        # ---- constant / input loads -------------------------------------
        # One big sprayed DMA per tensor (partitions fan out over the DMA
        # ports); q-path tensors first so the first matmul group can start,
        # with the y/k-path and v-path streams on separate issue queues.
        xT = [cpool.tile([P, N], MM_DT, name=f"xT{c}") for c in range(CCH)]
        yT = [cpool.tile([P, N], MM_DT, name=f"yT{c}") for c in range(CCH)]
        wq = [cpool.tile([P, C], MM_DT, name=f"wq{c}") for c in range(CCH)]
        wk = [cpool.tile([P, C], MM_DT, name=f"wk{c}") for c in range(CCH)]
        wv = [cpool.tile([P, C], MM_DT, name=f"wv{c}") for c in range(CCH)]
        bq_sb = cpool.tile([P, CCH], F32, name="bq_sb")
        bk_sb = cpool.tile([P, CCH], F32, name="bk_sb")
        # q-path on the sync (HWDGE) queue, fine-grained so the first
        # projection group can start after ~1.25 MB:
        for c in range(CCH):
            nc.sync.dma_start(wq[c][:], d["wqT"][c * P : (c + 1) * P, :])
        for h in range(2):
            for c in range(CCH):
                nc.sync.dma_start(
                    xT[c][:, h * 512 : (h + 1) * 512],
                    d["xT"][c * P : (c + 1) * P, h * 512 : (h + 1) * 512],
                )
        nc.sync.dma_start(bq_sb[:], d["bq"].rearrange("(c p) -> p c", p=P))
        # k/v-path + constants on the gpsimd queue (covered by q compute)
        bvb = cpool.tile([P, C], F32, name="bvb")
        bvxb = cpool.tile([P, C], F32, name="bvxb")
        consts = cpool.tile([P, 2], F32, name="consts")
        ident = cpool.tile([P, P], BF16, name="ident")
        for c in range(CCH):
            nc.gpsimd.dma_start(wk[c][:], d["wkT"][c * P : (c + 1) * P, :])
        for h in range(2):
            for c in range(CCH):
                nc.gpsimd.dma_start(
                    yT[c][:, h * 512 : (h + 1) * 512],
                    d["yT"][c * P : (c + 1) * P, h * 512 : (h + 1) * 512],
                )
        nc.gpsimd.dma_start(bk_sb[:], d["bk"].rearrange("(c p) -> p c", p=P))
        for c in range(CCH):
            nc.gpsimd.dma_start(wv[c][:], d["wvT"][c * P : (c + 1) * P, :])
        nc.gpsimd.dma_start(bvb[:], d["bvb"][:])
        nc.gpsimd.dma_start(bvxb[:], d["bvxb"][:])
        nc.gpsimd.dma_start(consts[:], d["consts"][:])
        nc.gpsimd.dma_start(ident[:], d["ident"][:])

        # stats tiles
        rmax = cpool.tile([P, 2 * NCH], F32, name="rmax")
        rmax_x = cpool.tile([P, NCH], F32, name="rmax_x")
        rmneg = cpool.tile([P, NCH], F32, name="rmneg")
        gmax0 = cpool.tile([P, 1], F32, name="gmax0")
        gmax1 = cpool.tile([P, 1], F32, name="gmax1")
        gneg = cpool.tile([P, 1], F32, name="gneg")
        ffp = cpool.tile([P, NCH], F32, name="ffp")
        fbf = cpool.tile([P, NCH], BF16, name="fbf")
        zx = cpool.tile([P, NCH], F32, name="zx")
        rx = cpool.tile([P, NCH], F32, name="rx")
        sx = cpool.tile([P, NCH], F32, name="sx")
        zy = cpool.tile([P, NCH], F32, name="zy")
        sy = cpool.tile([P, NCH], F32, name="sy")

        e_pool = ctx.enter_context(tc.tile_pool(name="epool", bufs=1))
        E = [e_pool.tile([P, N], F32, name=f"E{i}") for i in range(NCH)]
        v_pool = ctx.enter_context(tc.tile_pool(name="vpool", bufs=1))
        vxbf = [v_pool.tile([P, C], BF16, name=f"vxbf{r}") for r in range(NCH)]
        vybf = [v_pool.tile([P, C], BF16, name=f"vybf{r}") for r in range(NCH)]
        vxb = [v_pool.tile([P, C], F32, name=f"vxb{r}") for r in range(NCH)]

        # ---- q/k projections (fp32), transposed layout [c_out, i] -------
        with tc.tile_pool(name="qkpool", bufs=1) as qk_pool:
            qT = [qk_pool.tile([P, N], MM_DT, name=f"qT{c}") for c in range(CCH)]
            kT = [qk_pool.tile([P, N], MM_DT, name=f"kT{c}") for c in range(CCH)]
            for h in range(2):
                for co in range(CCH):
                    ps = psum_mm.tile([P, 512], F32, tag="mmps")
                    for ci in range(CCH):
                        nc.tensor.matmul(
                            ps[:],
                            wq[ci][:, co * P : (co + 1) * P],
                            xT[ci][:, h * 512 : (h + 1) * 512],
                            start=(ci == 0),
                            stop=(ci == CCH - 1),
                        )
                    if co % 2 == 0:
                        nc.scalar.activation(
                            qT[co][:, h * 512 : (h + 1) * 512], ps[:],
                            AF.Identity, bias=bq_sb[:, co : co + 1],
                        )
                    else:
                        nc.vector.tensor_scalar_add(
                            qT[co][:, h * 512 : (h + 1) * 512], ps[:],
                            bq_sb[:, co : co + 1],
                        )
            for h in range(2):
                for co in range(CCH):
                    ps = psum_mm.tile([P, 512], F32, tag="mmps")
                    for ci in range(CCH):
                        nc.tensor.matmul(
                            ps[:],
                            wk[ci][:, co * P : (co + 1) * P],
                            yT[ci][:, h * 512 : (h + 1) * 512],
                            start=(ci == 0),
                            stop=(ci == CCH - 1),
                        )
                    if co % 2 == 0:
                        nc.scalar.activation(
                            kT[co][:, h * 512 : (h + 1) * 512], ps[:],
                            AF.Identity, bias=bk_sb[:, co : co + 1],
                        )
                    else:
                        nc.vector.tensor_scalar_add(
                            kT[co][:, h * 512 : (h + 1) * 512], ps[:],
                            bk_sb[:, co : co + 1],
                        )

            # ---- energy E = q @ k.T (fp32), + row maxes -----------------
            for i in range(NCH):
                for h in range(2):
                    ps = psum_mm.tile([P, 512], F32, tag="mmps")
                    for cc in range(CCH):
                        nc.tensor.matmul(
                            ps[:],
                            qT[cc][:, i * P : (i + 1) * P],
                            kT[cc][:, h * 512 : (h + 1) * 512],
                            start=(cc == 0),
                            stop=(cc == CCH - 1),
                        )
                    nc.scalar.copy(E[i][:, h * 512 : (h + 1) * 512], ps[:])
                    nc.vector.reduce_max(
                        rmax[:, 2 * i + h : 2 * i + h + 1], ps[:], axis=AX
                    )

        # ---- softmax shift stats ----------------------------------------
        # P'[i,j] = exp(E[i,j] - rmax[i] + OFF)   (bf16, row-softmax scaled
        #           by e^OFF so cross-extreme entries stay in normal range)
        # f[i]    = exp(rmax[i] - gmax + OFF)     (folds the row shift back
        #           out for the column-softmax / out_y path)
        nc.vector.reduce_max(
            rmax_x[:], rmax[:].rearrange("p (i h) -> p i h", h=2), axis=AX
        )
        nc.vector.tensor_scalar(
            rmneg[:], rmax_x[:], -1.0, OFF, op0=ALU.mult, op1=ALU.add
        )
        nc.vector.reduce_max(gmax0[:], rmax[:], axis=AX)
        nc.gpsimd.partition_all_reduce(
            gmax1[:], gmax0[:], channels=P, reduce_op=bass_isa.ReduceOp.max
        )
        nc.vector.tensor_scalar(
            gneg[:], gmax1[:], -1.0, OFF, op0=ALU.mult, op1=ALU.add
        )
        nc.scalar.activation(fbf[:], rmax_x[:], AF.Exp, bias=gneg[:, 0:1])
        nc.vector.tensor_copy(ffp[:], fbf[:])  # f32 view for DVE scalar use

        # ---- value projections (fp32 in, bf16 rhs copies) ----------------
        # Emitted after E so the PE stays busy during the exp phase.
        for r in range(NCH):
            ps = psum_mm.tile([P, 512], F32, tag="mmps")
            for ci in range(CCH):
                nc.tensor.matmul(
                    ps[:], xT[ci][:, r * P : (r + 1) * P], wv[ci][:],
                    start=(ci == 0), stop=(ci == CCH - 1),
                )
            nc.scalar.copy(vxbf[r][:], ps[:])
            # vxb = (1+beta)*vx_nb + (1+beta+beta^2)*bv
            nc.vector.scalar_tensor_tensor(
                vxb[r][:], ps[:], consts[:, 0:1], bvxb[:],
                op0=ALU.mult, op1=ALU.add,
            )
        for r in range(NCH):
            ps = psum_mm.tile([P, 512], F32, tag="mmps")
            for ci in range(CCH):
                nc.tensor.matmul(
                    ps[:], yT[ci][:, r * P : (r + 1) * P], wv[ci][:],
                    start=(ci == 0), stop=(ci == CCH - 1),
                )
            # vy' = f[i] * vy_nb  (per-partition scale)
            nc.vector.tensor_scalar_mul(vybf[r][:], ps[:], ffp[:, r : r + 1])

        # ---- exp + out_x normalizers ------------------------------------
        p_pool = ctx.enter_context(tc.tile_pool(name="ppool", bufs=1))
        praw = [p_pool.tile([P, N], BF16, name=f"praw{i}") for i in range(NCH)]
        prawT = [p_pool.tile([P, N], BF16, name=f"prawT{j}") for j in range(NCH)]

        for i in range(NCH):
            nc.scalar.activation(
                praw[i][:], E[i][:], AF.Exp,
                bias=rmneg[:, i : i + 1], scale=1.0,
                accum_out=zx[:, i : i + 1],
            )
        nc.vector.reciprocal(rx[:], zx[:])
        nc.vector.tensor_scalar_mul(sx[:], rx[:], consts[:, 1:2])  # * beta^2

        # out_y normalizers: zy[j] = sum_i P'[i,j] * f[i] via f-vector matmuls
        zy_ps_pool = ctx.enter_context(
            tc.tile_pool(name="zyps", bufs=2, space="PSUM")
        )
        for jt in range(NCH):
            zps = zy_ps_pool.tile([P, 1], F32, tag="zyps")
            for ic in range(NCH):
                nc.tensor.matmul(
                    zps[:], praw[ic][:, jt * P : (jt + 1) * P],
                    fbf[:, ic : ic + 1],
                    start=(ic == 0), stop=(ic == NCH - 1),
                )
            nc.scalar.copy(zy[:, jt : jt + 1], zps[:])
        nc.vector.reciprocal(sy[:], zy[:])

        # ---- transposes + attention matmuls -----------------------------
        psum_tr = ctx.enter_context(tc.tile_pool(name="psum_tr", bufs=2, space="PSUM"))
        for t in range(NCH):
            # transpose row-chunk t of P into column pieces of prawT
            for jc in range(NCH):
                pst = psum_tr.tile([P, P], BF16, tag="trps")
                nc.tensor.transpose(
                    pst[:], praw[t][:, jc * P : (jc + 1) * P], ident[:]
                )
                nc.vector.tensor_copy(prawT[jc][:, t * P : (t + 1) * P], pst[:])

            # out_y tile t: contraction over i via praw directly
            ps = psum_mm.tile([P, 512], F32, tag="mmps")
            for ic in range(NCH):
                nc.tensor.matmul(
                    ps[:], praw[ic][:, t * P : (t + 1) * P], vybf[ic][:],
                    start=(ic == 0), stop=(ic == NCH - 1),
                )
            oy = out_pool.tile([P, C], F32, tag="oy")
            nc.vector.scalar_tensor_tensor(
                oy[:], ps[:], sy[:, t : t + 1], bvb[:],
                op0=ALU.mult, op1=ALU.add,
            )
            nc.sync.dma_start(d["out_y"][t * P : (t + 1) * P, :], oy[:])

            # out_x tile t: needs prawT[jc][:, t*P:...] for all jc = the
            # transposes of this t-iteration
            ps = psum_mm.tile([P, 512], F32, tag="mmps")
            for jc in range(NCH):
                nc.tensor.matmul(
                    ps[:], prawT[jc][:, t * P : (t + 1) * P], vxbf[jc][:],
                    start=(jc == 0), stop=(jc == NCH - 1),
                )
            ox = out_pool.tile([P, C], F32, tag="ox")
            nc.vector.scalar_tensor_tensor(
                ox[:], ps[:], sx[:, t : t + 1], vxb[t][:],
                op0=ALU.mult, op1=ALU.add,
            )
            nc.sync.dma_start(d["out_x"][t * P : (t + 1) * P, :], ox[:])


_CACHE = {}


def _build():
    if "nc" in _CACHE:
        return _CACHE["nc"]
    nc = bacc.Bacc(
        "TRN2", target_bir_lowering=False, debug=False, enable_asserts=False,
        num_devices=B,
    )
    d = {}
    d["xT"] = nc.dram_tensor("xT", [C, N], MM_DT, kind="ExternalInput").ap()
    d["yT"] = nc.dram_tensor("yT", [C, N], MM_DT, kind="ExternalInput").ap()
    d["wqT"] = nc.dram_tensor("wqT", [C, C], MM_DT, kind="ExternalInput").ap()
    d["wkT"] = nc.dram_tensor("wkT", [C, C], MM_DT, kind="ExternalInput").ap()
    d["wvT"] = nc.dram_tensor("wvT", [C, C], MM_DT, kind="ExternalInput").ap()
    d["bq"] = nc.dram_tensor("bq", [C], F32, kind="ExternalInput").ap()
    d["bk"] = nc.dram_tensor("bk", [C], F32, kind="ExternalInput").ap()
    d["bvb"] = nc.dram_tensor("bvb", [P, C], F32, kind="ExternalInput").ap()
    d["bvxb"] = nc.dram_tensor("bvxb", [P, C], F32, kind="ExternalInput").ap()
    d["consts"] = nc.dram_tensor("consts", [P, 2], F32, kind="ExternalInput").ap()
    d["ident"] = nc.dram_tensor("ident", [P, P], BF16, kind="ExternalInput").ap()
    d["out_x"] = nc.dram_tensor("out_x", [N, C], F32, kind="ExternalOutput").ap()
    d["out_y"] = nc.dram_tensor("out_y", [N, C], F32, kind="ExternalOutput").ap()

    with tile.TileContext(nc) as tc:
        _emit(nc, tc, d)
    nc.compile()
    _CACHE["nc"] = nc
    return nc


LAST_EXEC_NS = None


def kernel(x, y, Wq, bq, Wk, bk, Wv, bv, beta):
    global LAST_EXEC_NS
    nc = _build()

    x = np.asarray(x, np.float32)
    y = np.asarray(y, np.float32)
    beta_f = float(np.asarray(beta).reshape(-1)[0])
    c1 = 1.0 + beta_f
    c2 = beta_f * beta_f
    wqT = np.ascontiguousarray(np.asarray(Wq, np.float32).T)
    wkT = np.ascontiguousarray(np.asarray(Wk, np.float32).T)
    wvT = np.ascontiguousarray(np.asarray(Wv, np.float32).T)
    bq = np.ascontiguousarray(np.asarray(bq, np.float32))
    bk = np.ascontiguousarray(np.asarray(bk, np.float32))
    bv = np.asarray(bv, np.float32)
    bvb = np.tile(bv[None, :], (P, 1))
    bvxb = np.tile(((1.0 + beta_f + beta_f * beta_f) * bv)[None, :], (P, 1))
    consts = np.tile(np.array([[c1, c2]], np.float32), (P, 1))
    ident = np.eye(P, dtype=ml_dtypes.bfloat16)

    shared = {
        "wqT": wqT, "wkT": wkT, "wvT": wvT, "bq": bq, "bk": bk,
        "bvb": np.ascontiguousarray(bvb), "bvxb": np.ascontiguousarray(bvxb),
        "consts": np.ascontiguousarray(consts), "ident": ident,
    }
    in_maps = []
    for b in range(B):
        m = dict(shared)
        m["xT"] = np.ascontiguousarray(x[b].T)
        m["yT"] = np.ascontiguousarray(y[b].T)
        in_maps.append(m)

    trace = os.environ.get("KERNEL_TRACE", "0") == "1"
    res = run_bass_kernel_spmd(nc, in_maps, core_ids=list(range(B)), trace=trace)
    LAST_EXEC_NS = res.exec_time_ns

    out_x = np.stack([np.asarray(res.results[b]["out_x"]) for b in range(B)])
    out_y = np.stack([np.asarray(res.results[b]["out_y"]) for b in range(B)])
    return out_x, out_y


# revision 19
# speedup vs baseline: 1.0728x; 1.0240x over previous
"""Trainium2 Bass kernel for nn_Cross_Attn (B=8, N=1024, C=512).

Sharding: data-parallel over batch B across the 8 NeuronCores (one batch
element per core); the three [C,C] projection weights and beta are
replicated to every core.

Per-core math (batch element b):
  q = x @ Wq.T + bq ; k = y @ Wk.T + bk ; v* = {x,y} @ Wv.T + bv
  E[i,j] = q[i].k[j]
  out_x = beta^2 * softmax_row(E) @ vx + (1+beta) * vx     (reference applies
          the "beta*out+vx" residual twice)
  out_y = softmax_row(E.T) @ vy

Both softmaxes are served by ONE exponentiation with a single global shift
(gmax = max E): P = exp(E - gmax).  Row sums of P give the out_x
normalizer; row sums of P.T give the out_y normalizer, and P / P.T are
exactly the stationary matmul operands the two attention contractions
need.  Biases are folded in via "attention rows sum to 1":
  attn @ (v_nb + 1*bv) = attn @ v_nb + bv.

Precision: projections for q/k and the QK^T energy run in fp32 on the PE
(softmax logits have std ~22, bf16 there would corrupt the attention
weights); the post-softmax value path runs in bf16 (errors are averaged
by the convex attention combination).
"""

import os

import numpy as np
import ml_dtypes

import concourse.bacc as bacc
import concourse.bass as bass
import concourse.bass_isa as bass_isa
import concourse.mybir as mybir
import concourse.tile as tile
from concourse.bass_utils import run_bass_kernel_spmd

B, N, C = 8, 1024, 512
P = 128
NCH = N // P  # 8 chunks of token rows
CCH = C // P  # 4 chunks of channels
F32 = mybir.dt.float32
BF16 = mybir.dt.bfloat16
AX = mybir.AxisListType.X
ALU = mybir.AluOpType
AF = mybir.ActivationFunctionType
OFF = 35.0  # softmax range-centering offset (see _emit)
F32R = mybir.dt.float32r
# dtype used for the q/k/E/v projection matmul operands: float32 is exact
# but runs the PE at 1/4 rate; float32r uses the replicated-fp32 PE mode
# (full rate for free-dim >= 256) with reduced mantissa on hardware.
MM_DT = F32R if os.environ.get("KERNEL_MM_DT", "f32r") == "f32r" else F32



def _emit(nc, tc, d):
    """Emit the per-core kernel IR. `d` maps dram tensor name -> AP."""
    from contextlib import ExitStack

    with ExitStack() as ctx:
        cpool = ctx.enter_context(tc.tile_pool(name="const", bufs=1))
        psum_mm = ctx.enter_context(tc.tile_pool(name="psum_mm", bufs=4, space="PSUM"))
        out_pool = ctx.enter_context(tc.tile_pool(name="outs", bufs=4))

        # ---- constant / input loads -------------------------------------
        # One big sprayed DMA per tensor (partitions fan out over the DMA
        # ports); q-path tensors first so the first matmul group can start,
        # with the y/k-path and v-path streams on separate issue queues.
        xT = [cpool.tile([P, N], MM_DT, name=f"xT{c}") for c in range(CCH)]
        yT = [cpool.tile([P, N], MM_DT, name=f"yT{c}") for c in range(CCH)]
        wq = [cpool.tile([P, C], MM_DT, name=f"wq{c}") for c in range(CCH)]
        wk = [cpool.tile([P, C], MM_DT, name=f"wk{c}") for c in range(CCH)]
        wv = [cpool.tile([P, C], MM_DT, name=f"wv{c}") for c in range(CCH)]
        bq_sb = cpool.tile([P, CCH], F32, name="bq_sb")
        bk_sb = cpool.tile([P, CCH], F32, name="bk_sb")
        # q-path on the sync (HWDGE) queue, weights first so the first
        # projection group can start as soon as possible:
        for c in range(CCH):
            nc.sync.dma_start(wq[c][:], d["wqT"][c * P : (c + 1) * P, :])
        for h in range(2):
            for c in range(CCH):
                nc.sync.dma_start(
                    xT[c][:, h * 512 : (h + 1) * 512],
                    d["xT"][c * P : (c + 1) * P, h * 512 : (h + 1) * 512],
                )
        nc.sync.dma_start(bq_sb[:], d["bq"].rearrange("(c p) -> p c", p=P))
        # k/v-path + constants on the gpsimd queue (covered by q compute)
        bvb = cpool.tile([P, C], F32, name="bvb")
        bvxb = cpool.tile([P, C], F32, name="bvxb")
        consts = cpool.tile([P, 2], F32, name="consts")
        for c in range(CCH):
            nc.gpsimd.dma_start(wk[c][:], d["wkT"][c * P : (c + 1) * P, :])
        for h in range(2):
            for c in range(CCH):
                nc.gpsimd.dma_start(
                    yT[c][:, h * 512 : (h + 1) * 512],
                    d["yT"][c * P : (c + 1) * P, h * 512 : (h + 1) * 512],
                )
        nc.gpsimd.dma_start(bk_sb[:], d["bk"].rearrange("(c p) -> p c", p=P))
        for c in range(CCH):
            nc.gpsimd.dma_start(wv[c][:], d["wvT"][c * P : (c + 1) * P, :])
        nc.gpsimd.dma_start(bvb[:], d["bvb"][:])
        nc.gpsimd.dma_start(bvxb[:], d["bvxb"][:])
        nc.gpsimd.dma_start(consts[:], d["consts"][:])

        # stats tiles
        rmax = cpool.tile([P, 2 * NCH], F32, name="rmax")
        rmax_x = cpool.tile([P, NCH], F32, name="rmax_x")
        rmneg = cpool.tile([P, NCH], F32, name="rmneg")
        gmax0 = cpool.tile([P, 1], F32, name="gmax0")
        gmax1 = cpool.tile([P, 1], F32, name="gmax1")
        gneg = cpool.tile([P, 1], F32, name="gneg")
        ffp = cpool.tile([P, NCH], F32, name="ffp")
        fbf = cpool.tile([P, NCH], BF16, name="fbf")
        zx = cpool.tile([P, NCH], F32, name="zx")
        rx = cpool.tile([P, NCH], F32, name="rx")
        sx = cpool.tile([P, NCH], F32, name="sx")
        zy = cpool.tile([P, NCH], F32, name="zy")
        sy = cpool.tile([P, NCH], F32, name="sy")

        e_pool = ctx.enter_context(tc.tile_pool(name="epool", bufs=1))
        E = [e_pool.tile([P, N], F32, name=f"E{i}") for i in range(NCH)]
        v_pool = ctx.enter_context(tc.tile_pool(name="vpool", bufs=1))
        vxbf = [v_pool.tile([P, C], BF16, name=f"vxbf{r}") for r in range(NCH)]
        vybf = [v_pool.tile([P, C], BF16, name=f"vybf{r}") for r in range(NCH)]
        vxb = [v_pool.tile([P, C], F32, name=f"vxb{r}") for r in range(NCH)]

        # ---- q/k projections (fp32), transposed layout [c_out, i] -------
        with tc.tile_pool(name="qkpool", bufs=1) as qk_pool:
            qT = [qk_pool.tile([P, N], MM_DT, name=f"qT{c}") for c in range(CCH)]
            kT = [qk_pool.tile([P, N], MM_DT, name=f"kT{c}") for c in range(CCH)]
            for h in range(2):
                for co in range(CCH):
                    ps = psum_mm.tile([P, 512], F32, tag="mmps")
                    for ci in range(CCH):
                        nc.tensor.matmul(
                            ps[:],
                            wq[ci][:, co * P : (co + 1) * P],
                            xT[ci][:, h * 512 : (h + 1) * 512],
                            start=(ci == 0),
                            stop=(ci == CCH - 1),
                        )
                    if co % 2 == 0:
                        nc.scalar.activation(
                            qT[co][:, h * 512 : (h + 1) * 512], ps[:],
                            AF.Identity, bias=bq_sb[:, co : co + 1],
                        )
                    else:
                        nc.vector.tensor_scalar_add(
                            qT[co][:, h * 512 : (h + 1) * 512], ps[:],
                            bq_sb[:, co : co + 1],
                        )
            for h in range(2):
                for co in range(CCH):
                    ps = psum_mm.tile([P, 512], F32, tag="mmps")
                    for ci in range(CCH):
                        nc.tensor.matmul(
                            ps[:],
                            wk[ci][:, co * P : (co + 1) * P],
                            yT[ci][:, h * 512 : (h + 1) * 512],
                            start=(ci == 0),
                            stop=(ci == CCH - 1),
                        )
                    if co % 2 == 0:
                        nc.scalar.activation(
                            kT[co][:, h * 512 : (h + 1) * 512], ps[:],
                            AF.Identity, bias=bk_sb[:, co : co + 1],
                        )
                    else:
                        nc.vector.tensor_scalar_add(
                            kT[co][:, h * 512 : (h + 1) * 512], ps[:],
                            bk_sb[:, co : co + 1],
                        )

            # ---- energy E = q @ k.T (fp32), + row maxes -----------------
            for i in range(NCH):
                for h in range(2):
                    ps = psum_mm.tile([P, 512], F32, tag="mmps")
                    for cc in range(CCH):
                        nc.tensor.matmul(
                            ps[:],
                            qT[cc][:, i * P : (i + 1) * P],
                            kT[cc][:, h * 512 : (h + 1) * 512],
                            start=(cc == 0),
                            stop=(cc == CCH - 1),
                        )
                    nc.scalar.copy(E[i][:, h * 512 : (h + 1) * 512], ps[:])
                    nc.vector.reduce_max(
                        rmax[:, 2 * i + h : 2 * i + h + 1], ps[:], axis=AX
                    )

        # ---- softmax shift stats ----------------------------------------
        # P'[i,j] = exp(E[i,j] - rmax[i] + OFF)   (bf16, row-softmax scaled
        #           by e^OFF so cross-extreme entries stay in normal range)
        # f[i]    = exp(rmax[i] - gmax + OFF)     (folds the row shift back
        #           out for the column-softmax / out_y path)
        nc.vector.reduce_max(
            rmax_x[:], rmax[:].rearrange("p (i h) -> p i h", h=2), axis=AX
        )
        nc.vector.tensor_scalar(
            rmneg[:], rmax_x[:], -1.0, OFF, op0=ALU.mult, op1=ALU.add
        )
        nc.vector.reduce_max(gmax0[:], rmax[:], axis=AX)
        nc.gpsimd.partition_all_reduce(
            gmax1[:], gmax0[:], channels=P, reduce_op=bass_isa.ReduceOp.max
        )
        nc.vector.tensor_scalar(
            gneg[:], gmax1[:], -1.0, OFF, op0=ALU.mult, op1=ALU.add
        )
        nc.scalar.activation(fbf[:], rmax_x[:], AF.Exp, bias=gneg[:, 0:1])
        nc.vector.tensor_copy(ffp[:], fbf[:])  # f32 view for DVE scalar use

        # ---- value projections (fp32 in, bf16 rhs copies) ----------------
        # Emitted after E so the PE stays busy during the exp phase.
        for r in range(NCH):
            ps = psum_mm.tile([P, 512], F32, tag="mmps")
            for ci in range(CCH):
                nc.tensor.matmul(
                    ps[:], xT[ci][:, r * P : (r + 1) * P], wv[ci][:],
                    start=(ci == 0), stop=(ci == CCH - 1),
                )
            nc.scalar.copy(vxbf[r][:], ps[:])
            # vxb = (1+beta)*vx_nb + (1+beta+beta^2)*bv
            nc.vector.scalar_tensor_tensor(
                vxb[r][:], ps[:], consts[:, 0:1], bvxb[:],
                op0=ALU.mult, op1=ALU.add,
            )
        for r in range(NCH):
            ps = psum_mm.tile([P, 512], F32, tag="mmps")
            for ci in range(CCH):
                nc.tensor.matmul(
                    ps[:], yT[ci][:, r * P : (r + 1) * P], wv[ci][:],
                    start=(ci == 0), stop=(ci == CCH - 1),
                )
            # vy' = f[i] * vy_nb  (per-partition scale)
            nc.vector.tensor_scalar_mul(vybf[r][:], ps[:], ffp[:, r : r + 1])

        # ---- exp + out_x normalizers ------------------------------------
        p_pool = ctx.enter_context(tc.tile_pool(name="ppool", bufs=1))
        praw_t = p_pool.tile([P, NCH, N], BF16, name="praw_t")
        prawT_t = p_pool.tile([P, NCH, N], BF16, name="prawT_t")
        praw = [praw_t[:, i, :] for i in range(NCH)]

        for i in range(NCH):
            nc.scalar.activation(
                praw[i][:], E[i][:], AF.Exp,
                bias=rmneg[:, i : i + 1], scale=1.0,
                accum_out=zx[:, i : i + 1],
            )
        nc.vector.reciprocal(rx[:], zx[:])
        nc.vector.tensor_scalar_mul(sx[:], rx[:], consts[:, 1:2])  # * beta^2

        # out_y normalizers: zy[j] = sum_i P'[i,j] * f[i] via f-vector matmuls
        zy_ps_pool = ctx.enter_context(
            tc.tile_pool(name="zyps", bufs=2, space="PSUM")
        )
        for jt in range(NCH):
            zps = zy_ps_pool.tile([P, 1], F32, tag="zyps")
            for ic in range(NCH):
                nc.tensor.matmul(
                    zps[:], praw[ic][:, jt * P : (jt + 1) * P],
                    fbf[:, ic : ic + 1],
                    start=(ic == 0), stop=(ic == NCH - 1),
                )
            nc.scalar.copy(zy[:, jt : jt + 1], zps[:])
        nc.vector.reciprocal(sy[:], zy[:])

        # ---- transpose P via the DMA crossbar (frees PE + DVE) ----------
        # prawT_t[p, jc, ic*128+f] = praw[ic-chunk][f, jc*128+p]
        for ic in range(NCH):
            nc.sync.dma_start(
                prawT_t[:, :, ic * P : (ic + 1) * P], praw_t[:, ic, :],
                transpose=True,
            )
        prawT = [prawT_t[:, j, :] for j in range(NCH)]

        # ---- attention matmuls ------------------------------------------
        for t in range(NCH):
            # out_y tile t: contraction over i via praw directly
            ps = psum_mm.tile([P, 512], F32, tag="mmps")
            for ic in range(NCH):
                nc.tensor.matmul(
                    ps[:], praw[ic][:, t * P : (t + 1) * P], vybf[ic][:],
                    start=(ic == 0), stop=(ic == NCH - 1),
                )
            oy = out_pool.tile([P, C], F32, tag="oy")
            nc.vector.scalar_tensor_tensor(
                oy[:], ps[:], sy[:, t : t + 1], bvb[:],
                op0=ALU.mult, op1=ALU.add,
            )
            nc.scalar.dma_start(d["out_y"][t * P : (t + 1) * P, :], oy[:])

        for t in range(NCH):
            ps = psum_mm.tile([P, 512], F32, tag="mmps")
            for jc in range(NCH):
                nc.tensor.matmul(
                    ps[:], prawT[jc][:, t * P : (t + 1) * P], vxbf[jc][:],
                    start=(jc == 0), stop=(jc == NCH - 1),
                )
            ox = out_pool.tile([P, C], F32, tag="ox")
            nc.vector.scalar_tensor_tensor(
                ox[:], ps[:], sx[:, t : t + 1], vxb[t][:],
                op0=ALU.mult, op1=ALU.add,
            )
            nc.scalar.dma_start(d["out_x"][t * P : (t + 1) * P, :], ox[:])


_CACHE = {}


def _build():
    if "nc" in _CACHE:
        return _CACHE["nc"]
    nc = bacc.Bacc(
        "TRN2", target_bir_lowering=False, debug=False, enable_asserts=False,
        num_devices=B,
    )
    d = {}
    d["xT"] = nc.dram_tensor("xT", [C, N], MM_DT, kind="ExternalInput").ap()
    d["yT"] = nc.dram_tensor("yT", [C, N], MM_DT, kind="ExternalInput").ap()
    d["wqT"] = nc.dram_tensor("wqT", [C, C], MM_DT, kind="ExternalInput").ap()
    d["wkT"] = nc.dram_tensor("wkT", [C, C], MM_DT, kind="ExternalInput").ap()
    d["wvT"] = nc.dram_tensor("wvT", [C, C], MM_DT, kind="ExternalInput").ap()
    d["bq"] = nc.dram_tensor("bq", [C], F32, kind="ExternalInput").ap()
    d["bk"] = nc.dram_tensor("bk", [C], F32, kind="ExternalInput").ap()
    d["bvb"] = nc.dram_tensor("bvb", [P, C], F32, kind="ExternalInput").ap()
    d["bvxb"] = nc.dram_tensor("bvxb", [P, C], F32, kind="ExternalInput").ap()
    d["consts"] = nc.dram_tensor("consts", [P, 2], F32, kind="ExternalInput").ap()
    d["out_x"] = nc.dram_tensor("out_x", [N, C], F32, kind="ExternalOutput").ap()
    d["out_y"] = nc.dram_tensor("out_y", [N, C], F32, kind="ExternalOutput").ap()

    with tile.TileContext(nc) as tc:
        _emit(nc, tc, d)
    nc.compile()
    _CACHE["nc"] = nc
    return nc


LAST_EXEC_NS = None


def kernel(x, y, Wq, bq, Wk, bk, Wv, bv, beta):
    global LAST_EXEC_NS
    nc = _build()

    x = np.asarray(x, np.float32)
    y = np.asarray(y, np.float32)
    beta_f = float(np.asarray(beta).reshape(-1)[0])
    c1 = 1.0 + beta_f
    c2 = beta_f * beta_f
    wqT = np.ascontiguousarray(np.asarray(Wq, np.float32).T)
    wkT = np.ascontiguousarray(np.asarray(Wk, np.float32).T)
    wvT = np.ascontiguousarray(np.asarray(Wv, np.float32).T)
    bq = np.ascontiguousarray(np.asarray(bq, np.float32))
    bk = np.ascontiguousarray(np.asarray(bk, np.float32))
    bv = np.asarray(bv, np.float32)
    bvb = np.tile(bv[None, :], (P, 1))
    bvxb = np.tile(((1.0 + beta_f + beta_f * beta_f) * bv)[None, :], (P, 1))
    consts = np.tile(np.array([[c1, c2]], np.float32), (P, 1))

    shared = {
        "wqT": wqT, "wkT": wkT, "wvT": wvT, "bq": bq, "bk": bk,
        "bvb": np.ascontiguousarray(bvb), "bvxb": np.ascontiguousarray(bvxb),
        "consts": np.ascontiguousarray(consts),
    }
    in_maps = []
    for b in range(B):
        m = dict(shared)
        m["xT"] = np.ascontiguousarray(x[b].T)
        m["yT"] = np.ascontiguousarray(y[b].T)
        in_maps.append(m)

    trace = os.environ.get("KERNEL_TRACE", "0") == "1"
    res = run_bass_kernel_spmd(nc, in_maps, core_ids=list(range(B)), trace=trace)
    LAST_EXEC_NS = res.exec_time_ns

    out_x = np.stack([np.asarray(res.results[b]["out_x"]) for b in range(B)])
    out_y = np.stack([np.asarray(res.results[b]["out_y"]) for b in range(B)])
    return out_x, out_y
